# revision 47
# baseline (speedup 1.0000x reference)
"""BasicNCA (neural cellular automaton) Trainium2 kernel, 8-core SPMD.

Reference computation (per step, 32 steps):
  p  = depthwise3x3(s, [identity, sobel_x, sobel_y])   # (B, 3C, H, W)
  h  = relu(w1 @ p + b1)                               # (B, 64, H, W)
  d  = w2 @ h + b2                                     # (B, C, H, W)
  s += d * (mask < 0.5)

Implementation notes:
- Perception conv + first 1x1 conv fuse into one effective 3x3 conv with
  weights Weff[o, c, dy, dx]; computed on the PE as 9 shifted "tap" matmuls
  accumulating in PSUM.
- All matmuls run in fp8e4 DoubleRow mode (0.5 cycles/row in the cost
  model): each tap's DR pair is (A, B) where Weff*SW ~ A + B (A = fp8
  round, B = fp8 of the residual), with the rhs pair dim a stride-0
  broadcast of the same fp8 state view. This keeps weight-quantization
  error ~16x below plain fp8 (final rel err ~0.009 vs the 0.02 gate).
  Layer 2 is the same trick with w2*SW2 and fp8 h; DR needs the lhsT
  pair-dim step % 16 == 0, hence M padded 120->128 there.
- State: fp32 master s_sb (DVE-only) + fp8 shadow s8 refreshed per step by
  2x-rate DVE tensor_scalar copies; ACT's relu applies the descale and b1
  for free. Fire masks are host-precomputed (m<0.5) as fp8 {0,1}.
- Sharding: core i handles batch i//2, H-half i%2, with a 32-row taper of
  redundant compute instead of per-step halo exchange between cores
  (validity shrinks 1 row/step; 32 steps consume exactly the margin).
- A core's 96-row slab splits into 4 sub-slabs of 24 rows on the 4 SBUF
  partition quadrants (channels 0-23 of each). Tap matmuls process slab
  PAIRS (K=56 block lhsT, both slabs' h in the output partition halves,
  tile_position=(base,0)); layer 2 runs K=128 over both slabs' h with
  per-slab 24-column output blocks.
- The per-step schedule is static and latency-tuned: chunk-pair order
  (0,1,3,2) so a step never opens on a chunk adjacent to the previous
  step's last chunk; each chunk's layer2+mask+update is emitted one tap
  group late so the PE never waits on ACT relu; the s8 refresh is split by
  row range and emitted only after every reader of the old value (tap
  reads span rows 6k-2..6k+6 — NOTE the dx-shifted taps poke 2 elements
  into the preceding row, which the range tracker sees); the last chunk's
  finish is pipelined into the next step's first tap-group slot; the
  critical rows-19..23 shadow is fused as s8 = fp8(s_old + u) so it skips
  the fp32 master-add. Intra-core sub-slab halos go on the idle gpsimd
  DMA queue, off the critical path.
"""

import sys

sys.path.insert(0, "/opt/trn_rl_repo")

import numpy as np
import ml_dtypes

import concourse.bass as bass
import concourse.bacc as bacc
import concourse.tile as tile
import concourse.mybir as mybir

dt = mybir.dt

B, C, H, W = 4, 24, 128, 128
HID = 64
FIRE_RATE = 0.5
N_CORES = 8

SH = 96            # slab rows per core (64 own + 32 taper)
SR = 24            # rows per sub-slab (one partition quadrant)
FW = W + 2         # padded row width (130)
FR = SR + 2        # frame rows per sub-slab (26)
FRAME_OFF = 4      # leading guard elems so tap offset -1 stays in-bounds
FRAME = FR * FW    # 3380
S_FREE = FRAME_OFF + FRAME + 4
COMP = SR * FW     # 3120 compact free size (real rows 0..23)
NCH = 390          # chunk = 3 rows
NCHUNK = COMP // NCH  # 8

SS = 1.0           # fp8 state shadow scale (e4m3 is relative-precision)
SW = 256.0         # fp8 tap weight scale
SH8 = 16.0         # fp8 h scale (relu output)
SW2 = 2048.0       # fp8 w2 scale

LAST_EXEC_NS = None
_cache = {}

F8 = ml_dtypes.float8_e4m3


def _taps():
    # correlation taps: out(y, x) = sum_{dy,dx} in(y+dy-1, x+dx-1) * k[dy, dx]
    return [(dy, dx) for dy in range(3) for dx in range(3)]


def _pair0(ap2d):
    """[K, N] view -> [K, 2, N] with a stride-0 pair dim (DoubleRow rhs)."""
    k, n = ap2d.shape
    return ap2d.unsqueeze(1).broadcast_to([k, 2, n])


def _pair(ap2d, stride):
    """[K, N] view -> [K, 2, N] with pair-dim stride (two shifted taps)."""
    if stride == 0:
        return _pair0(ap2d)
    v = ap2d.unsqueeze(1)
    c = v.copy()
    V = type(c.ap)
    c.ap = V([list(v.ap[0]), [stride, 2], list(v.ap[2])])
    return c


def _build_program(steps, apply_b2, repeats=1):
    nc = bacc.Bacc("TRN2", target_bir_lowering=False, debug=False,
                   num_devices=N_CORES)

    s_d = nc.dram_tensor("s0", [128, S_FREE], dt.float32, kind="ExternalInput")
    s8_d = nc.dram_tensor("s80", [128, S_FREE], dt.float8e4,
                          kind="ExternalInput")
    m_d = nc.dram_tensor("masks", [steps, 128, COMP], dt.float8e4,
                         kind="ExternalInput")
    tw8_d = nc.dram_tensor("tw8", [128, 5 * 256], dt.float8e4,
                           kind="ExternalInput")
    w2b_d = nc.dram_tensor("w2b", [128, 2 * 256], dt.float8e4,
                           kind="ExternalInput")
    b2r_d = nc.dram_tensor("b2r", [128, 1], dt.float32, kind="ExternalInput")
    b1_d = nc.dram_tensor("b1v", [128, 1], dt.float32, kind="ExternalInput")
    out_d = nc.dram_tensor("out", [128, SR * W], dt.float32,
                           kind="ExternalOutput")

    DR = mybir.MatmulPerfMode.DoubleRow

    with tile.TileContext(nc) as tc:
        with tc.tile_pool(name="persist", bufs=1) as pp, \
             tc.tile_pool(name="mpool", bufs=2) as mpool, \
             tc.tile_pool(name="hsb", bufs=4) as hsbp, \
             tc.tile_pool(name="upool", bufs=2) as upool, \
             tc.tile_pool(name="hps", bufs=3, space="PSUM") as hps_pool, \
             tc.tile_pool(name="dps", bufs=1, space="PSUM") as dps_pool:

            s_sb = pp.tile([128, S_FREE], dt.float32)
            s8 = pp.tile([128, S_FREE], dt.float8e4)
            tw8 = pp.tile([128, 5 * 256], dt.float8e4)
            w2b = pp.tile([128, 2 * 256], dt.float8e4)
            b2r = pp.tile([128, 1], dt.float32)
            b1v = pp.tile([128, 1], dt.float32)

            nc.sync.dma_start(tw8[:], tw8_d[:])
            nc.gpsimd.dma_start(s_sb[:], s_d[:])
            nc.scalar.dma_start(s8[:], s8_d[:])
            nc.sync.dma_start(w2b[:], w2b_d[:])
            nc.sync.dma_start(b2r[:], b2r_d[:])
            nc.sync.dma_start(b1v[:], b1_d[:])

            def halo_top():
                # top halos: sub g+1 frame row 0 <- sub g real row 23,
                # on the otherwise-idle gpsimd DMA queue (off critical path)
                for g in range(3):
                    nc.gpsimd.dma_start(
                        s8[32 * (g + 1):32 * (g + 1) + 24,
                           FRAME_OFF:FRAME_OFF + FW],
                        s8[32 * g:32 * g + 24,
                           FRAME_OFF + 24 * FW:FRAME_OFF + 24 * FW + FW],
                    )

            def halo_bot():
                # bottom halos: sub g frame row 25 <- sub g+1 real row 0
                for g in range(3):
                    nc.gpsimd.dma_start(
                        s8[32 * g:32 * g + 24,
                           FRAME_OFF + 25 * FW:FRAME_OFF + 25 * FW + FW],
                        s8[32 * (g + 1):32 * (g + 1) + 24,
                           FRAME_OFF + FW:FRAME_OFF + FW + FW],
                    )

            # DR slot plan: 5 matmuls per chunk cover the 9 taps — pairs
            # (0,1), (2,3), (5,6), (7,8) ride the two DR slots as two
            # shifted views (plain fp8, their quantization residuals are
            # small and mostly cancel), while the dominant center tap gets
            # the (A, B) compensated pair on a stride-0 view.
            # (dy, dx) of the first tap of each matmul and the pair stride:
            TAP_MMS = [
                ((0, 0), 2),            # taps (0,0)+(0,2)
                ((0, 1), 2 * FW),       # taps (0,1)+(2,1)
                ((1, 1), 0),            # center (A, B)
                ((1, 0), 2),            # taps (1,0)+(1,2)
                ((2, 0), 2),            # taps (2,0)+(2,2)
            ]

            def tap_group(k, p, split_relu=False):
                # 5-matmul fp8 DoubleRow chain for chunk pair k, slab pair
                # p, then relu+b1 (with the fp8 descale) into an fp8 h tile
                base = 64 * p
                hps = hps_pool.tile([128, 1024], dt.float32, tag="hps")
                for cc in range(2):
                    c = 2 * k + cc
                    for ti, ((dy, dx), stride) in enumerate(TAP_MMS):
                        off = FRAME_OFF + (3 * c + dy) * FW + dx - 1
                        nc.tensor.matmul(
                            hps[:, 512 * cc:512 * cc + NCH],
                            tw8[base:base + 56,
                                256 * ti:256 * ti + 256].rearrange(
                                    "p (two m) -> p two m", two=2),
                            _pair(s8[base:base + 56, off:off + NCH], stride),
                            start=(ti == 0), stop=(ti == 4),
                            tile_position=(base, 0),
                            perf_mode=DR,
                        )
                hsb = hsbp.tile([128, 2 * NCH], dt.float8e4, tag=f"hsb{p}")
                if split_relu:
                    # per-sub-chunk relu starts at the chain midpoint, so a
                    # layer2 that must run right after this group (the
                    # step-tail pipeline) does not wait a full relu latency
                    for cc in range(2):
                        nc.scalar.activation(
                            hsb[:, NCH * cc:NCH * cc + NCH],
                            hps[:, 512 * cc:512 * cc + NCH],
                            mybir.ActivationFunctionType.Relu,
                            bias=b1v[:, 0:1],
                            scale=SH8 / (SS * SW),
                        )
                else:
                    nc.scalar.activation(
                        hsb[:].rearrange("p (b x) -> p b x", x=NCH),
                        hps[:].rearrange("p (b x) -> p b x", b=2)[:, :, 0:NCH],
                        mybir.ActivationFunctionType.Relu,
                        bias=b1v[:, 0:1],
                        scale=SH8 / (SS * SW),
                    )
                return hsb

            def finish_chunk(k, hsb_list, m_sb, u_sb, fuse=None,
                             add_eng=None):
                # layer2 (fp8 DR, compensated), fire-mask, s update for
                # chunk pair k
                dps = dps_pool.tile([128, 1024], dt.float32, tag="dps")
                for cc in range(2):
                    for p in range(2):
                        # M padded to 128 (out rows 120-127 junk): DoubleRow
                        # requires the lhsT pair-dim step % 16 == 0
                        nc.tensor.matmul(
                            dps[0:128, 512 * cc:512 * cc + NCH],
                            w2b[:, 256 * p:256 * p + 256].rearrange(
                                "p (two m) -> p two m", two=2),
                            _pair0(hsb_list[p][:, NCH * cc:NCH * cc + NCH]),
                            start=(p == 0), stop=(p == 1),
                            perf_mode=DR,
                        )
                if apply_b2:
                    # delta += b2*SW2*SH8 (per-partition scalar), in psum
                    nc.vector.tensor_scalar_add(
                        dps[0:120].rearrange(
                            "p (b x) -> p b x", b=2)[:, :, 0:NCH],
                        dps[0:120].rearrange(
                            "p (b x) -> p b x", b=2)[:, :, 0:NCH],
                        b2r[0:120, 0:1],
                    )
                # u = m01 * delta / (SW2*SH8)
                nc.vector.scalar_tensor_tensor(
                    u_sb[0:120, 780 * k:780 * k + 780].rearrange(
                        "p (b x) -> p b x", x=NCH),
                    m_sb[0:120, 780 * k:780 * k + 780].rearrange(
                        "p (b x) -> p b x", x=NCH),
                    1.0 / (SW2 * SH8),
                    dps[0:120].rearrange("p (b x) -> p b x", b=2)[:, :, 0:NCH],
                    mybir.AluOpType.mult,
                    mybir.AluOpType.mult,
                )
                # fused critical shadow piece: s8 = fp8(s_old + u) straight
                # off u, skipping the wait on the fp32 master add
                if fuse is not None:
                    r0, r1 = fuse
                    fa = FRAME_OFF + FW + FW * r0
                    fn = FW * (r1 - r0 + 1)
                    nc.vector.tensor_add(
                        s8[0:120, fa:fa + fn],
                        s_sb[0:120, fa:fa + fn],
                        u_sb[0:120, FW * r0:FW * r0 + fn],
                    )
                # s += u (fp32 master); slack-tolerant chunks go on the
                # otherwise-idle gpsimd engine to unload the DVE
                a = FRAME_OFF + FW + 780 * k
                (add_eng or nc.vector).tensor_add(
                    s_sb[0:120, a:a + 780],
                    s_sb[0:120, a:a + 780],
                    u_sb[0:120, 780 * k:780 * k + 780],
                )

            def finish_cc(k, cc, hsb_list, m_sb, u_sb):
                # one sub-chunk (3 rows) of layer2 + mask-mult + fused
                # shadow + Pool add, so the step tail pipelines at finer
                # grain under the next step's tap groups
                dps = dps_pool.tile([128, 1024], dt.float32, tag="dps")
                for p in range(2):
                    nc.tensor.matmul(
                        dps[0:128, 512 * cc:512 * cc + NCH],
                        w2b[:, 256 * p:256 * p + 256].rearrange(
                            "p (two m) -> p two m", two=2),
                        _pair0(hsb_list[p][:, NCH * cc:NCH * cc + NCH]),
                        start=(p == 0), stop=(p == 1),
                        perf_mode=DR,
                    )
                if apply_b2:
                    nc.vector.tensor_scalar_add(
                        dps[0:120, 512 * cc:512 * cc + NCH],
                        dps[0:120, 512 * cc:512 * cc + NCH],
                        b2r[0:120, 0:1],
                    )
                u0 = 780 * k + NCH * cc
                nc.vector.scalar_tensor_tensor(
                    u_sb[0:120, u0:u0 + NCH],
                    m_sb[0:120, u0:u0 + NCH],
                    1.0 / (SW2 * SH8),
                    dps[0:120, 512 * cc:512 * cc + NCH],
                    mybir.AluOpType.mult,
                    mybir.AluOpType.mult,
                )
                r0 = 6 * k + 3 * cc
                a = FRAME_OFF + FW + FW * r0
                if cc == 0:
                    # rows 10-12 gate the next step's second tap group:
                    # fused shadow for chunk2's own rows, copies for the
                    # already-updated neighbors
                    nc.vector.tensor_add(
                        s8[0:120, a:a + 3 * FW], s_sb[0:120, a:a + 3 * FW],
                        u_sb[0:120, u0:u0 + NCH])
                    shadow_rows(10, 11)
                else:
                    nc.vector.tensor_add(
                        s8[0:120, a:a + 3 * FW], s_sb[0:120, a:a + 3 * FW],
                        u_sb[0:120, u0:u0 + NCH])
                    shadow_rows(18, 18)
                nc.gpsimd.tensor_add(
                    s_sb[0:120, a:a + 3 * FW], s_sb[0:120, a:a + 3 * FW],
                    u_sb[0:120, u0:u0 + NCH])

            def shadow_rows(r0, r1):
                # refresh fp8 shadow for real rows r0..r1 inclusive
                a = FRAME_OFF + FW + FW * r0
                n = FW * (r1 - r0 + 1)
                nc.vector.tensor_scalar_mul(
                    s8[0:120, a:a + n], s_sb[0:120, a:a + n], SS)

            # Static per-step schedule. Chunk-pair order (0, 3, 1, 2):
            # consecutive steps never start on a chunk adjacent to the
            # previous step's last chunk, layer2 of chunk j is delayed one
            # tap group so the PE never waits on ACT relu, and the shadow
            # refresh is split by row range so each piece is emitted only
            # after every reader of the old value (tap groups of adjacent
            # chunks, which overlap by one row) has been issued.
            # tiny warmup matmuls right after the weight load start the
            # cost model's PE p-state ramp clock during the state DMAs
            warm = dps_pool.tile([128, 1024], dt.float32, tag="dps")
            for _ in range(3):
                nc.tensor.matmul(warm[:, 0:128], tw8[0:56, 0:128],
                                 tw8[0:56, 128:256], start=True, stop=True)

            pending = pending_b = None
            for t in range(steps * repeats):
                t = t % steps
                m_sb = mpool.tile([128, COMP], dt.float8e4, tag="m")
                nc.sync.dma_start(m_sb[:], m_d[t])
                u_sb = upool.tile([128, COMP], dt.float32, tag="u")

                hs = {}
                # it0 — the previous step's tail (finish of its chunk 2 and
                # the late shadow pieces) is emitted between this step's
                # first two tap groups, so its relu wait and DVE chain hide
                # under PE work instead of stalling the step boundary
                hs[0] = [tap_group(0, 0)]
                if pending is not None:
                    pending()
                hs[0].append(tap_group(0, 1))
                if pending_b is not None:
                    pending_b()
                # it1
                hs[1] = [tap_group(1, 0)]
                finish_chunk(0, hs[0], m_sb, u_sb, add_eng=nc.gpsimd)
                hs[1].append(tap_group(1, 1))
                # it2 (chunk k's taps read rows 6k-2 .. 6k+6, so a shadow
                # piece for row r must follow the tap groups of every chunk
                # k with 6k-2 <= r <= 6k+6)
                hs[3] = [tap_group(3, 0)]
                finish_chunk(1, hs[1], m_sb, u_sb, add_eng=nc.gpsimd)
                shadow_rows(0, 4)
                shadow_rows(5, 9)
                hs[3].append(tap_group(3, 1))
                # it3
                hs[2] = [tap_group(2, 0, split_relu=True)]
                finish_chunk(3, hs[3], m_sb, u_sb, fuse=(19, 23),
                             add_eng=nc.gpsimd)
                halo_top()
                halo_bot()
                hs[2].append(tap_group(2, 1, split_relu=True))

                hs2, msb_t, usb_t = hs[2], m_sb, u_sb

                def make_pending(cc, hs2=hs2, m_sb=msb_t, u_sb=usb_t):
                    def p():
                        finish_cc(2, cc, hs2, m_sb, u_sb)
                    return p

                pending = make_pending(0)
                pending_b = make_pending(1)

            pending()
            pending_b()

            # write back real pixels (frame rows 1..24, cols 1..128)
            a0 = FRAME_OFF + FW + 1
            nc.sync.dma_start(
                out_d[:].rearrange("p (r x) -> p r x", x=W),
                s_sb[:, a0:a0 + SR * FW].rearrange(
                    "p (r x) -> p r x", x=FW)[:, :, 0:W],
            )

    nc.compile()
    return nc


def _prep_weights(w1, b1, w2, b2):
    sx = np.array([[-1, 0, 1], [-2, 0, 2], [-1, 0, 1]], np.float32) / 8.0
    sy = sx.T.copy()
    ident = np.zeros((3, 3), np.float32)
    ident[1, 1] = 1.0
    # Weff[o, c, dy, dx]
    weff = (np.einsum("oc,yx->ocyx", w1[:, 0::3], ident)
            + np.einsum("oc,yx->ocyx", w1[:, 1::3], sx)
            + np.einsum("oc,yx->ocyx", w1[:, 2::3], sy)).astype(np.float32)

    # fp8 DR tap lhsT, 5 matmul slots per chunk (must match TAP_MMS):
    # slots 0,1,3,4 carry two different taps' fp8 weights; slot 2 carries
    # the center tap's (A, B) compensation pair. Block structure: rows
    # 0-23 -> h of even slab at out cols 0-63, rows 32-55 -> h of odd slab
    # at out cols 64-127 (same again at partition base 64).
    def q8w(dy, dx):
        return (weff[:, :, dy, dx].T * SW).astype(F8).astype(np.float32)

    cA = weff[:, :, 1, 1].T * SW
    cAq = cA.astype(F8).astype(np.float32)
    cBq = (cA - cAq).astype(F8).astype(np.float32)
    slot_pairs = [
        (q8w(0, 0), q8w(0, 2)),
        (q8w(0, 1), q8w(2, 1)),
        (cAq, cBq),
        (q8w(1, 0), q8w(1, 2)),
        (q8w(2, 0), q8w(2, 2)),
    ]
    tw8 = np.zeros((128, 5 * 256), np.float32)
    for ti, (w0, w1_) in enumerate(slot_pairs):
        for p in range(2):
            base = 64 * p
            o = 256 * ti
            tw8[base:base + 24, o:o + 64] = w0
            tw8[base + 32:base + 56, o + 64:o + 128] = w0
            tw8[base:base + 24, o + 128:o + 192] = w1_
            tw8[base + 32:base + 56, o + 192:o + 256] = w1_
    tw8 = tw8.astype(F8)

    # layer2 lhsT per pair: K=128 (both h halves), M=120 with 24-col blocks
    # placing each slab's delta on its partition quadrant. fp8 DR with (A,B)
    # compensation: slot0 = A = fp8(w2*SW2), slot1 = B = fp8(w2*SW2 - A).
    w2s = w2.T * SW2
    w2A = w2s.astype(F8).astype(np.float32)
    w2B = (w2s - w2A).astype(F8).astype(np.float32)
    w2b = np.zeros((128, 2 * 256), np.float32)
    for p in range(2):
        ge, go = 2 * p, 2 * p + 1
        o = 256 * p
        w2b[0:64, o + 32 * ge:o + 32 * ge + 24] = w2A
        w2b[64:128, o + 32 * go:o + 32 * go + 24] = w2A
        w2b[0:64, o + 128 + 32 * ge:o + 128 + 32 * ge + 24] = w2B
        w2b[64:128, o + 128 + 32 * go:o + 128 + 32 * go + 24] = w2B
    w2b = w2b.astype(F8)

    b2r = np.zeros((128, 1), np.float32)
    b1v = np.zeros((128, 1), np.float32)
    for g in range(4):
        b2r[32 * g:32 * g + 24, 0] = b2 * SW2 * SH8
    b1v[0:64, 0] = b1
    b1v[64:128, 0] = b1
    return tw8, w2b, b2r, b1v


def _prep_state(state):
    """state (B, C, H, W) -> per-core [128, S_FREE] framed slabs (+fp8)."""
    bufs = []
    for core in range(N_CORES):
        b = core // 2
        top = (core % 2) == 0
        r0 = 0 if top else H - SH
        buf = np.zeros((128, S_FREE), np.float32)
        for ch in range(C):
            full = np.zeros((SH + 2, FW), np.float32)
            full[1:SH + 1, 1:W + 1] = state[b, ch, r0:r0 + SH, :]
            if r0 > 0:
                full[0, 1:W + 1] = state[b, ch, r0 - 1, :]
            if r0 + SH < H:
                full[SH + 1, 1:W + 1] = state[b, ch, r0 + SH, :]
            for g in range(4):
                fr = full[g * SR:g * SR + FR, :]
                buf[32 * g + ch, FRAME_OFF:FRAME_OFF + FRAME] = fr.reshape(-1)
        bufs.append((buf, (buf * SS).astype(F8)))
    return bufs


def _prep_masks(masks):
    """masks (S, B, 1, H, W) -> per-core [S, 128, COMP] fp8 {0,1} fire."""
    S = masks.shape[0]
    bufs = []
    for core in range(N_CORES):
        b = core // 2
        top = (core % 2) == 0
        r0 = 0 if top else H - SH
        mb = np.zeros((S, 128, COMP), F8)
        fire = (masks[:, b, 0, r0:r0 + SH, :] < FIRE_RATE).astype(np.float32)
        mrows = np.zeros((S, SH, FW), np.float32)
        mrows[:, :, 1:W + 1] = fire
        for g in range(4):
            seg = mrows[:, g * SR:(g + 1) * SR, :].reshape(S, COMP)
            mb[:, 32 * g:32 * g + C, :] = seg[:, None, :].astype(F8)
        bufs.append(mb)
    return bufs


def kernel(state, w1, b1, w2, b2, masks):
    state = np.asarray(state)
    w1, b1 = np.asarray(w1), np.asarray(b1)
    w2, b2 = np.asarray(w2), np.asarray(b2)
    masks = np.asarray(masks)
    import os as _os
    steps = masks.shape[0]
    apply_b2 = bool(np.any(b2 != 0))
    repeats = int(_os.environ.get("NCA_REPEAT", "1"))
    key = ("prog", steps, apply_b2, repeats)
    if key not in _cache:
        _cache[key] = _build_program(steps, apply_b2, repeats)
    nc = _cache[key]

    from concourse.bass_utils import run_bass_kernel_spmd

    tw8, w2b, b2r, b1v = _prep_weights(w1, b1, w2, b2)
    s_bufs = _prep_state(state)
    m_bufs = _prep_masks(masks)

    in_maps = []
    for core in range(N_CORES):
        in_maps.append({
            "s0": s_bufs[core][0],
            "s80": s_bufs[core][1],
            "masks": m_bufs[core],
            "tw8": tw8,
            "w2b": w2b,
            "b2r": b2r,
            "b1v": b1v,
        })

    import os
    trace = bool(os.environ.get("NCA_TRACE"))
    kw = {}
    if trace:
        kw["trace"] = True
        if os.environ.get("NCA_TRACE_DIR"):
            kw["tmpdir"] = os.environ["NCA_TRACE_DIR"]
    res = run_bass_kernel_spmd(nc, in_maps, list(range(N_CORES)), **kw)
    global LAST_EXEC_NS
    LAST_EXEC_NS = res.exec_time_ns

    out = np.zeros((B, C, H, W), np.float32)
    for core in range(N_CORES):
        o = res.results[core]["out"]  # [128, SR*W]
        b = core // 2
        top = (core % 2) == 0
        r0 = 0 if top else H - SH
        own0 = 0 if top else H // 2
        for g in range(4):
            rows = o[32 * g:32 * g + 24].reshape(C, SR, W)
            g0 = r0 + g * SR
            lo = max(g0, own0)
            hi = min(g0 + SR, own0 + H // 2)
            if lo < hi:
                out[b, :, lo:hi, :] = rows[:, lo - g0:hi - g0, :]
    return out


# revision 48
# speedup vs baseline: 1.0264x; 1.0264x over previous
"""BasicNCA (neural cellular automaton) Trainium2 kernel, 8-core SPMD.

Reference computation (per step, 32 steps):
  p  = depthwise3x3(s, [identity, sobel_x, sobel_y])   # (B, 3C, H, W)
  h  = relu(w1 @ p + b1)                               # (B, 64, H, W)
  d  = w2 @ h + b2                                     # (B, C, H, W)
  s += d * (mask < 0.5)

Implementation notes:
- Perception conv + first 1x1 conv fuse into one effective 3x3 conv with
  weights Weff[o, c, dy, dx]; computed on the PE as 9 shifted "tap" matmuls
  accumulating in PSUM.
- All matmuls run in fp8e4 DoubleRow mode (0.5 cycles/row in the cost
  model): each tap's DR pair is (A, B) where Weff*SW ~ A + B (A = fp8
  round, B = fp8 of the residual), with the rhs pair dim a stride-0
  broadcast of the same fp8 state view. This keeps weight-quantization
  error ~16x below plain fp8 (final rel err ~0.009 vs the 0.02 gate).
  Layer 2 is the same trick with w2*SW2 and fp8 h; DR needs the lhsT
  pair-dim step % 16 == 0, hence M padded 120->128 there.
- State: fp32 master s_sb (DVE-only) + fp8 shadow s8 refreshed per step by
  2x-rate DVE tensor_scalar copies; ACT's relu applies the descale and b1
  for free. Fire masks are host-precomputed (m<0.5) as fp8 {0,1}.
- Sharding: core i handles batch i//2, H-half i%2, with a 32-row taper of
  redundant compute instead of per-step halo exchange between cores
  (validity shrinks 1 row/step; 32 steps consume exactly the margin).
- A core's 96-row slab splits into 4 sub-slabs of 24 rows on the 4 SBUF
  partition quadrants (channels 0-23 of each). Tap matmuls process slab
  PAIRS (K=56 block lhsT, both slabs' h in the output partition halves,
  tile_position=(base,0)); layer 2 runs K=128 over both slabs' h with
  per-slab 24-column output blocks.
- The per-step schedule is static and latency-tuned: chunk-pair order
  (0,1,3,2) so a step never opens on a chunk adjacent to the previous
  step's last chunk; each chunk's layer2+mask+update is emitted one tap
  group late so the PE never waits on ACT relu; the s8 refresh is split by
  row range and emitted only after every reader of the old value (tap
  reads span rows 6k-2..6k+6 — NOTE the dx-shifted taps poke 2 elements
  into the preceding row, which the range tracker sees); the last chunk's
  finish is pipelined into the next step's first tap-group slot; the
  critical rows-19..23 shadow is fused as s8 = fp8(s_old + u) so it skips
  the fp32 master-add. Intra-core sub-slab halos go on the idle gpsimd
  DMA queue, off the critical path.
"""

import sys

sys.path.insert(0, "/opt/trn_rl_repo")

import numpy as np
import ml_dtypes

import concourse.bass as bass
import concourse.bacc as bacc
import concourse.tile as tile
import concourse.mybir as mybir

dt = mybir.dt

B, C, H, W = 4, 24, 128, 128
HID = 64
FIRE_RATE = 0.5
N_CORES = 8

SH = 96            # slab rows per core (64 own + 32 taper)
SR = 24            # rows per sub-slab (one partition quadrant)
FW = W + 2         # padded row width (130)
FR = SR + 2        # frame rows per sub-slab (26)
FRAME_OFF = 4      # leading guard elems so tap offset -1 stays in-bounds
FRAME = FR * FW    # 3380
S_FREE = FRAME_OFF + FRAME + 4
COMP = SR * FW     # 3120 compact free size (real rows 0..23)
NCH = 390          # chunk = 3 rows
NCHUNK = COMP // NCH  # 8

SS = 1.0           # fp8 state shadow scale (e4m3 is relative-precision)
SW = 256.0         # fp8 tap weight scale
SH8 = 16.0         # fp8 h scale (relu output)
SW2 = 2048.0       # fp8 w2 scale

LAST_EXEC_NS = None
_cache = {}

F8 = ml_dtypes.float8_e4m3


def _taps():
    # correlation taps: out(y, x) = sum_{dy,dx} in(y+dy-1, x+dx-1) * k[dy, dx]
    return [(dy, dx) for dy in range(3) for dx in range(3)]


def _pair0(ap2d):
    """[K, N] view -> [K, 2, N] with a stride-0 pair dim (DoubleRow rhs)."""
    k, n = ap2d.shape
    return ap2d.unsqueeze(1).broadcast_to([k, 2, n])


def _pair(ap2d, stride):
    """[K, N] view -> [K, 2, N] with pair-dim stride (two shifted taps)."""
    if stride == 0:
        return _pair0(ap2d)
    v = ap2d.unsqueeze(1)
    c = v.copy()
    V = type(c.ap)
    c.ap = V([list(v.ap[0]), [stride, 2], list(v.ap[2])])
    return c


def _build_program(steps, apply_b2, repeats=1):
    nc = bacc.Bacc("TRN2", target_bir_lowering=False, debug=False,
                   num_devices=N_CORES)

    s_d = nc.dram_tensor("s0", [128, S_FREE], dt.float32, kind="ExternalInput")
    s8_d = nc.dram_tensor("s80", [128, S_FREE], dt.float8e4,
                          kind="ExternalInput")
    m_d = nc.dram_tensor("masks", [steps, 128, COMP], dt.float8e4,
                         kind="ExternalInput")
    tw8_d = nc.dram_tensor("tw8", [128, 5 * 256], dt.float8e4,
                           kind="ExternalInput")
    w2b_d = nc.dram_tensor("w2b", [128, 2 * 256], dt.float8e4,
                           kind="ExternalInput")
    b2r_d = nc.dram_tensor("b2r", [128, 1], dt.float32, kind="ExternalInput")
    b1_d = nc.dram_tensor("b1v", [128, 1], dt.float32, kind="ExternalInput")
    out_d = nc.dram_tensor("out", [128, SR * W], dt.float32,
                           kind="ExternalOutput")

    DR = mybir.MatmulPerfMode.DoubleRow

    with tile.TileContext(nc) as tc:
        with tc.tile_pool(name="persist", bufs=1) as pp, \
             tc.tile_pool(name="mpool", bufs=2) as mpool, \
             tc.tile_pool(name="hsb", bufs=4) as hsbp, \
             tc.tile_pool(name="upool", bufs=2) as upool, \
             tc.tile_pool(name="hps", bufs=3, space="PSUM") as hps_pool, \
             tc.tile_pool(name="dps", bufs=1, space="PSUM") as dps_pool:

            s_sb = pp.tile([128, S_FREE], dt.float32)
            s8 = pp.tile([128, S_FREE], dt.float8e4)
            tw8 = pp.tile([128, 5 * 256], dt.float8e4)
            w2b = pp.tile([128, 2 * 256], dt.float8e4)
            b2r = pp.tile([128, 1], dt.float32)
            b1v = pp.tile([128, 1], dt.float32)

            nc.sync.dma_start(tw8[:], tw8_d[:])
            nc.gpsimd.dma_start(s_sb[:], s_d[:])
            nc.scalar.dma_start(s8[:], s8_d[:])
            nc.sync.dma_start(w2b[:], w2b_d[:])
            nc.sync.dma_start(b2r[:], b2r_d[:])
            nc.sync.dma_start(b1v[:], b1_d[:])

            def halo_top():
                # top halos: sub g+1 frame row 0 <- sub g real row 23,
                # on the otherwise-idle gpsimd DMA queue (off critical path)
                for g in range(3):
                    nc.gpsimd.dma_start(
                        s8[32 * (g + 1):32 * (g + 1) + 24,
                           FRAME_OFF:FRAME_OFF + FW],
                        s8[32 * g:32 * g + 24,
                           FRAME_OFF + 24 * FW:FRAME_OFF + 24 * FW + FW],
                    )

            def halo_bot():
                # bottom halos: sub g frame row 25 <- sub g+1 real row 0
                for g in range(3):
                    nc.gpsimd.dma_start(
                        s8[32 * g:32 * g + 24,
                           FRAME_OFF + 25 * FW:FRAME_OFF + 25 * FW + FW],
                        s8[32 * (g + 1):32 * (g + 1) + 24,
                           FRAME_OFF + FW:FRAME_OFF + FW + FW],
                    )

            # DR slot plan: 5 matmuls per chunk cover the 9 taps — pairs
            # (0,1), (2,3), (5,6), (7,8) ride the two DR slots as two
            # shifted views (plain fp8, their quantization residuals are
            # small and mostly cancel), while the dominant center tap gets
            # the (A, B) compensated pair on a stride-0 view.
            # (dy, dx) of the first tap of each matmul and the pair stride:
            TAP_MMS = [
                ((0, 0), 2),            # taps (0,0)+(0,2)
                ((0, 1), 2 * FW),       # taps (0,1)+(2,1)
                ((1, 1), 0),            # center (A, B)
                ((1, 0), 2),            # taps (1,0)+(1,2)
                ((2, 0), 2),            # taps (2,0)+(2,2)
            ]

            def tap_group(k, p, split_relu=False):
                # 5-matmul fp8 DoubleRow chain for chunk pair k, slab pair
                # p, then relu+b1 (with the fp8 descale) into an fp8 h tile
                base = 64 * p
                hps = hps_pool.tile([128, 1024], dt.float32, tag="hps")
                for cc in range(2):
                    c = 2 * k + cc
                    for ti, ((dy, dx), stride) in enumerate(TAP_MMS):
                        off = FRAME_OFF + (3 * c + dy) * FW + dx - 1
                        nc.tensor.matmul(
                            hps[:, 512 * cc:512 * cc + NCH],
                            tw8[base:base + 56,
                                256 * ti:256 * ti + 256].rearrange(
                                    "p (two m) -> p two m", two=2),
                            _pair(s8[base:base + 56, off:off + NCH], stride),
                            start=(ti == 0), stop=(ti == 4),
                            tile_position=(base, 0),
                            perf_mode=DR,
                        )
                hsb = hsbp.tile([128, 2 * NCH], dt.float8e4, tag=f"hsb{p}")
                if split_relu:
                    # per-sub-chunk relu starts at the chain midpoint, so a
                    # layer2 that must run right after this group (the
                    # step-tail pipeline) does not wait a full relu latency
                    for cc in range(2):
                        nc.scalar.activation(
                            hsb[:, NCH * cc:NCH * cc + NCH],
                            hps[:, 512 * cc:512 * cc + NCH],
                            mybir.ActivationFunctionType.Relu,
                            bias=b1v[:, 0:1],
                            scale=SH8 / (SS * SW),
                        )
                else:
                    nc.scalar.activation(
                        hsb[:].rearrange("p (b x) -> p b x", x=NCH),
                        hps[:].rearrange("p (b x) -> p b x", b=2)[:, :, 0:NCH],
                        mybir.ActivationFunctionType.Relu,
                        bias=b1v[:, 0:1],
                        scale=SH8 / (SS * SW),
                    )
                return hsb

            def finish_chunk(k, hsb_list, m_sb, u_sb, fuse=None,
                             add_eng=None):
                # layer2 (fp8 DR, compensated), fire-mask, s update for
                # chunk pair k
                dps = dps_pool.tile([128, 1024], dt.float32, tag="dps")
                for cc in range(2):
                    for p in range(2):
                        # M padded to 128 (out rows 120-127 junk): DoubleRow
                        # requires the lhsT pair-dim step % 16 == 0
                        nc.tensor.matmul(
                            dps[0:128, 512 * cc:512 * cc + NCH],
                            w2b[:, 256 * p:256 * p + 256].rearrange(
                                "p (two m) -> p two m", two=2),
                            _pair0(hsb_list[p][:, NCH * cc:NCH * cc + NCH]),
                            start=(p == 0), stop=(p == 1),
                            perf_mode=DR,
                        )
                if apply_b2:
                    # delta += b2*SW2*SH8 (per-partition scalar), in psum
                    nc.vector.tensor_scalar_add(
                        dps[0:120].rearrange(
                            "p (b x) -> p b x", b=2)[:, :, 0:NCH],
                        dps[0:120].rearrange(
                            "p (b x) -> p b x", b=2)[:, :, 0:NCH],
                        b2r[0:120, 0:1],
                    )
                # u = m01 * delta / (SW2*SH8)
                nc.vector.scalar_tensor_tensor(
                    u_sb[0:120, 780 * k:780 * k + 780].rearrange(
                        "p (b x) -> p b x", x=NCH),
                    m_sb[0:120, 780 * k:780 * k + 780].rearrange(
                        "p (b x) -> p b x", x=NCH),
                    1.0 / (SW2 * SH8),
                    dps[0:120].rearrange("p (b x) -> p b x", b=2)[:, :, 0:NCH],
                    mybir.AluOpType.mult,
                    mybir.AluOpType.mult,
                )
                # fused critical shadow piece: s8 = fp8(s_old + u) straight
                # off u, skipping the wait on the fp32 master add
                if fuse is not None:
                    r0, r1 = fuse
                    fa = FRAME_OFF + FW + FW * r0
                    fn = FW * (r1 - r0 + 1)
                    nc.vector.tensor_add(
                        s8[0:120, fa:fa + fn],
                        s_sb[0:120, fa:fa + fn],
                        u_sb[0:120, FW * r0:FW * r0 + fn],
                    )
                # s += u (fp32 master); slack-tolerant chunks go on the
                # otherwise-idle gpsimd engine to unload the DVE
                a = FRAME_OFF + FW + 780 * k
                (add_eng or nc.vector).tensor_add(
                    s_sb[0:120, a:a + 780],
                    s_sb[0:120, a:a + 780],
                    u_sb[0:120, 780 * k:780 * k + 780],
                )

            def shadow_rows(r0, r1):
                # refresh fp8 shadow for real rows r0..r1 inclusive
                a = FRAME_OFF + FW + FW * r0
                n = FW * (r1 - r0 + 1)
                nc.vector.tensor_scalar_mul(
                    s8[0:120, a:a + n], s_sb[0:120, a:a + n], SS)

            # Static per-step schedule. Chunk-pair order (0, 3, 1, 2):
            # consecutive steps never start on a chunk adjacent to the
            # previous step's last chunk, layer2 of chunk j is delayed one
            # tap group so the PE never waits on ACT relu, and the shadow
            # refresh is split by row range so each piece is emitted only
            # after every reader of the old value (tap groups of adjacent
            # chunks, which overlap by one row) has been issued.
            # tiny warmup matmuls right after the weight load start the
            # cost model's PE p-state ramp clock during the state DMAs
            warm = dps_pool.tile([128, 1024], dt.float32, tag="dps")
            for _ in range(3):
                nc.tensor.matmul(warm[:, 0:128], tw8[0:56, 0:128],
                                 tw8[0:56, 128:256], start=True, stop=True)

            pending = None
            for t in range(steps * repeats):
                t = t % steps
                m_sb = mpool.tile([128, COMP], dt.float8e4, tag="m")
                nc.sync.dma_start(m_sb[:], m_d[t])
                u_sb = upool.tile([128, COMP], dt.float32, tag="u")

                hs = {}
                # it0 — the previous step's tail (finish of its chunk 2 and
                # the late shadow pieces) is emitted between this step's
                # first two tap groups, so its relu wait and DVE chain hide
                # under PE work instead of stalling the step boundary
                hs[0] = [tap_group(0, 0)]
                if pending is not None:
                    pending()
                hs[0].append(tap_group(0, 1))
                # it1
                hs[1] = [tap_group(1, 0)]
                finish_chunk(0, hs[0], m_sb, u_sb, add_eng=nc.gpsimd)
                hs[1].append(tap_group(1, 1))
                # it2 (chunk k's taps read rows 6k-2 .. 6k+6, so a shadow
                # piece for row r must follow the tap groups of every chunk
                # k with 6k-2 <= r <= 6k+6)
                hs[3] = [tap_group(3, 0)]
                finish_chunk(1, hs[1], m_sb, u_sb, add_eng=nc.gpsimd)
                shadow_rows(0, 4)
                shadow_rows(5, 9)
                hs[3].append(tap_group(3, 1))
                # it3
                hs[2] = [tap_group(2, 0)]
                finish_chunk(3, hs[3], m_sb, u_sb, fuse=(19, 23),
                             add_eng=nc.gpsimd)
                halo_top()
                halo_bot()
                hs[2].append(tap_group(2, 1))

                hs2, msb_t, usb_t = hs[2], m_sb, u_sb

                def pending(hs2=hs2, m_sb=msb_t, u_sb=usb_t):
                    # rows 12-17 are chunk 2's own (pre-add) so they ride
                    # the fused s8=fp8(s_old+u) path; rows 10-11 and 18
                    # belong to already-updated chunks, plain copies
                    finish_chunk(2, hs2, m_sb, u_sb, fuse=(12, 17),
                                 add_eng=nc.gpsimd)
                    shadow_rows(10, 11)
                    shadow_rows(18, 18)

            pending()

            # write back real pixels (frame rows 1..24, cols 1..128)
            a0 = FRAME_OFF + FW + 1
            nc.sync.dma_start(
                out_d[:].rearrange("p (r x) -> p r x", x=W),
                s_sb[:, a0:a0 + SR * FW].rearrange(
                    "p (r x) -> p r x", x=FW)[:, :, 0:W],
            )

    nc.compile()
    return nc


def _prep_weights(w1, b1, w2, b2):
    sx = np.array([[-1, 0, 1], [-2, 0, 2], [-1, 0, 1]], np.float32) / 8.0
    sy = sx.T.copy()
    ident = np.zeros((3, 3), np.float32)
    ident[1, 1] = 1.0
    # Weff[o, c, dy, dx]
    weff = (np.einsum("oc,yx->ocyx", w1[:, 0::3], ident)
            + np.einsum("oc,yx->ocyx", w1[:, 1::3], sx)
            + np.einsum("oc,yx->ocyx", w1[:, 2::3], sy)).astype(np.float32)

    # fp8 DR tap lhsT, 5 matmul slots per chunk (must match TAP_MMS):
    # slots 0,1,3,4 carry two different taps' fp8 weights; slot 2 carries
    # the center tap's (A, B) compensation pair. Block structure: rows
    # 0-23 -> h of even slab at out cols 0-63, rows 32-55 -> h of odd slab
    # at out cols 64-127 (same again at partition base 64).
    def q8w(dy, dx):
        return (weff[:, :, dy, dx].T * SW).astype(F8).astype(np.float32)

    cA = weff[:, :, 1, 1].T * SW
    cAq = cA.astype(F8).astype(np.float32)
    cBq = (cA - cAq).astype(F8).astype(np.float32)
    slot_pairs = [
        (q8w(0, 0), q8w(0, 2)),
        (q8w(0, 1), q8w(2, 1)),
        (cAq, cBq),
        (q8w(1, 0), q8w(1, 2)),
        (q8w(2, 0), q8w(2, 2)),
    ]
    tw8 = np.zeros((128, 5 * 256), np.float32)
    for ti, (w0, w1_) in enumerate(slot_pairs):
        for p in range(2):
            base = 64 * p
            o = 256 * ti
            tw8[base:base + 24, o:o + 64] = w0
            tw8[base + 32:base + 56, o + 64:o + 128] = w0
            tw8[base:base + 24, o + 128:o + 192] = w1_
            tw8[base + 32:base + 56, o + 192:o + 256] = w1_
    tw8 = tw8.astype(F8)

    # layer2 lhsT per pair: K=128 (both h halves), M=120 with 24-col blocks
    # placing each slab's delta on its partition quadrant. fp8 DR with (A,B)
    # compensation: slot0 = A = fp8(w2*SW2), slot1 = B = fp8(w2*SW2 - A).
    w2s = w2.T * SW2
    w2A = w2s.astype(F8).astype(np.float32)
    w2B = (w2s - w2A).astype(F8).astype(np.float32)
    w2b = np.zeros((128, 2 * 256), np.float32)
    for p in range(2):
        ge, go = 2 * p, 2 * p + 1
        o = 256 * p
        w2b[0:64, o + 32 * ge:o + 32 * ge + 24] = w2A
        w2b[64:128, o + 32 * go:o + 32 * go + 24] = w2A
        w2b[0:64, o + 128 + 32 * ge:o + 128 + 32 * ge + 24] = w2B
        w2b[64:128, o + 128 + 32 * go:o + 128 + 32 * go + 24] = w2B
    w2b = w2b.astype(F8)

    b2r = np.zeros((128, 1), np.float32)
    b1v = np.zeros((128, 1), np.float32)
    for g in range(4):
        b2r[32 * g:32 * g + 24, 0] = b2 * SW2 * SH8
    b1v[0:64, 0] = b1
    b1v[64:128, 0] = b1
    return tw8, w2b, b2r, b1v


def _prep_state(state):
    """state (B, C, H, W) -> per-core [128, S_FREE] framed slabs (+fp8)."""
    bufs = []
    for core in range(N_CORES):
        b = core // 2
        top = (core % 2) == 0
        r0 = 0 if top else H - SH
        buf = np.zeros((128, S_FREE), np.float32)
        for ch in range(C):
            full = np.zeros((SH + 2, FW), np.float32)
            full[1:SH + 1, 1:W + 1] = state[b, ch, r0:r0 + SH, :]
            if r0 > 0:
                full[0, 1:W + 1] = state[b, ch, r0 - 1, :]
            if r0 + SH < H:
                full[SH + 1, 1:W + 1] = state[b, ch, r0 + SH, :]
            for g in range(4):
                fr = full[g * SR:g * SR + FR, :]
                buf[32 * g + ch, FRAME_OFF:FRAME_OFF + FRAME] = fr.reshape(-1)
        bufs.append((buf, (buf * SS).astype(F8)))
    return bufs


def _prep_masks(masks):
    """masks (S, B, 1, H, W) -> per-core [S, 128, COMP] fp8 {0,1} fire."""
    S = masks.shape[0]
    bufs = []
    for core in range(N_CORES):
        b = core // 2
        top = (core % 2) == 0
        r0 = 0 if top else H - SH
        mb = np.zeros((S, 128, COMP), F8)
        fire = (masks[:, b, 0, r0:r0 + SH, :] < FIRE_RATE).astype(np.float32)
        mrows = np.zeros((S, SH, FW), np.float32)
        mrows[:, :, 1:W + 1] = fire
        for g in range(4):
            seg = mrows[:, g * SR:(g + 1) * SR, :].reshape(S, COMP)
            mb[:, 32 * g:32 * g + C, :] = seg[:, None, :].astype(F8)
        bufs.append(mb)
    return bufs


def kernel(state, w1, b1, w2, b2, masks):
    state = np.asarray(state)
    w1, b1 = np.asarray(w1), np.asarray(b1)
    w2, b2 = np.asarray(w2), np.asarray(b2)
    masks = np.asarray(masks)
    import os as _os
    steps = masks.shape[0]
    apply_b2 = bool(np.any(b2 != 0))
    repeats = int(_os.environ.get("NCA_REPEAT", "1"))
    key = ("prog", steps, apply_b2, repeats)
    if key not in _cache:
        _cache[key] = _build_program(steps, apply_b2, repeats)
    nc = _cache[key]

    from concourse.bass_utils import run_bass_kernel_spmd

    tw8, w2b, b2r, b1v = _prep_weights(w1, b1, w2, b2)
    s_bufs = _prep_state(state)
    m_bufs = _prep_masks(masks)

    in_maps = []
    for core in range(N_CORES):
        in_maps.append({
            "s0": s_bufs[core][0],
            "s80": s_bufs[core][1],
            "masks": m_bufs[core],
            "tw8": tw8,
            "w2b": w2b,
            "b2r": b2r,
            "b1v": b1v,
        })

    import os
    trace = bool(os.environ.get("NCA_TRACE"))
    kw = {}
    if trace:
        kw["trace"] = True
        if os.environ.get("NCA_TRACE_DIR"):
            kw["tmpdir"] = os.environ["NCA_TRACE_DIR"]
    res = run_bass_kernel_spmd(nc, in_maps, list(range(N_CORES)), **kw)
    global LAST_EXEC_NS
    LAST_EXEC_NS = res.exec_time_ns

    out = np.zeros((B, C, H, W), np.float32)
    for core in range(N_CORES):
        o = res.results[core]["out"]  # [128, SR*W]
        b = core // 2
        top = (core % 2) == 0
        r0 = 0 if top else H - SH
        own0 = 0 if top else H // 2
        for g in range(4):
            rows = o[32 * g:32 * g + 24].reshape(C, SR, W)
            g0 = r0 + g * SR
            lo = max(g0, own0)
            hi = min(g0 + SR, own0 + H // 2)
            if lo < hi:
                out[b, :, lo:hi, :] = rows[:, lo - g0:hi - g0, :]
    return out


# revision 49
# speedup vs baseline: 1.0313x; 1.0047x over previous
"""BasicNCA (neural cellular automaton) Trainium2 kernel, 8-core SPMD.

Reference computation (per step, 32 steps):
  p  = depthwise3x3(s, [identity, sobel_x, sobel_y])   # (B, 3C, H, W)
  h  = relu(w1 @ p + b1)                               # (B, 64, H, W)
  d  = w2 @ h + b2                                     # (B, C, H, W)
  s += d * (mask < 0.5)

Implementation notes:
- Perception conv + first 1x1 conv fuse into one effective 3x3 conv with
  weights Weff[o, c, dy, dx]; computed on the PE as 9 shifted "tap" matmuls
  accumulating in PSUM.
- All matmuls run in fp8e4 DoubleRow mode (0.5 cycles/row in the cost
  model): each tap's DR pair is (A, B) where Weff*SW ~ A + B (A = fp8
  round, B = fp8 of the residual), with the rhs pair dim a stride-0
  broadcast of the same fp8 state view. This keeps weight-quantization
  error ~16x below plain fp8 (final rel err ~0.009 vs the 0.02 gate).
  Layer 2 is the same trick with w2*SW2 and fp8 h; DR needs the lhsT
  pair-dim step % 16 == 0, hence M padded 120->128 there.
- State: fp32 master s_sb (DVE-only) + fp8 shadow s8 refreshed per step by
  2x-rate DVE tensor_scalar copies; ACT's relu applies the descale and b1
  for free. Fire masks are host-precomputed (m<0.5) as fp8 {0,1}.
- Sharding: core i handles batch i//2, H-half i%2, with a 32-row taper of
  redundant compute instead of per-step halo exchange between cores
  (validity shrinks 1 row/step; 32 steps consume exactly the margin).
- A core's 96-row slab splits into 4 sub-slabs of 24 rows on the 4 SBUF
  partition quadrants (channels 0-23 of each). Tap matmuls process slab
  PAIRS (K=56 block lhsT, both slabs' h in the output partition halves,
  tile_position=(base,0)); layer 2 runs K=128 over both slabs' h with
  per-slab 24-column output blocks.
- The per-step schedule is static and latency-tuned: chunk-pair order
  (0,1,3,2) so a step never opens on a chunk adjacent to the previous
  step's last chunk; each chunk's layer2+mask+update is emitted one tap
  group late so the PE never waits on ACT relu; the s8 refresh is split by
  row range and emitted only after every reader of the old value (tap
  reads span rows 6k-2..6k+6 — NOTE the dx-shifted taps poke 2 elements
  into the preceding row, which the range tracker sees); the last chunk's
  finish is pipelined into the next step's first tap-group slot; the
  critical rows-19..23 shadow is fused as s8 = fp8(s_old + u) so it skips
  the fp32 master-add. Intra-core sub-slab halos go on the idle gpsimd
  DMA queue, off the critical path.
"""

import sys

sys.path.insert(0, "/opt/trn_rl_repo")

import numpy as np
import ml_dtypes

import concourse.bass as bass
import concourse.bacc as bacc
import concourse.tile as tile
import concourse.mybir as mybir

dt = mybir.dt

B, C, H, W = 4, 24, 128, 128
HID = 64
FIRE_RATE = 0.5
N_CORES = 8

SH = 96            # slab rows per core (64 own + 32 taper)
SR = 24            # rows per sub-slab (one partition quadrant)
FW = W + 2         # padded row width (130)
FR = SR + 2        # frame rows per sub-slab (26)
FRAME_OFF = 4      # leading guard elems so tap offset -1 stays in-bounds
FRAME = FR * FW    # 3380
S_FREE = FRAME_OFF + FRAME + 4
COMP = SR * FW     # 3120 compact free size (real rows 0..23)
NCH = 390          # chunk = 3 rows
NCHUNK = COMP // NCH  # 8

SS = 1.0           # fp8 state shadow scale (e4m3 is relative-precision)
SW = 256.0         # fp8 tap weight scale
SH8 = 16.0         # fp8 h scale (relu output)
SW2 = 2048.0       # fp8 w2 scale

LAST_EXEC_NS = None
_cache = {}

F8 = ml_dtypes.float8_e4m3


def _taps():
    # correlation taps: out(y, x) = sum_{dy,dx} in(y+dy-1, x+dx-1) * k[dy, dx]
    return [(dy, dx) for dy in range(3) for dx in range(3)]


def _pair0(ap2d):
    """[K, N] view -> [K, 2, N] with a stride-0 pair dim (DoubleRow rhs)."""
    k, n = ap2d.shape
    return ap2d.unsqueeze(1).broadcast_to([k, 2, n])


def _pair(ap2d, stride):
    """[K, N] view -> [K, 2, N] with pair-dim stride (two shifted taps)."""
    if stride == 0:
        return _pair0(ap2d)
    v = ap2d.unsqueeze(1)
    c = v.copy()
    V = type(c.ap)
    c.ap = V([list(v.ap[0]), [stride, 2], list(v.ap[2])])
    return c


def _build_program(steps, apply_b2, repeats=1):
    nc = bacc.Bacc("TRN2", target_bir_lowering=False, debug=False,
                   num_devices=N_CORES)

    s_d = nc.dram_tensor("s0", [128, S_FREE], dt.float32, kind="ExternalInput")
    s8_d = nc.dram_tensor("s80", [128, S_FREE], dt.float8e4,
                          kind="ExternalInput")
    m_d = nc.dram_tensor("masks", [steps, 128, COMP], dt.float8e4,
                         kind="ExternalInput")
    tw8_d = nc.dram_tensor("tw8", [128, 5 * 256], dt.float8e4,
                           kind="ExternalInput")
    w2b_d = nc.dram_tensor("w2b", [128, 2 * 256], dt.float8e4,
                           kind="ExternalInput")
    b2r_d = nc.dram_tensor("b2r", [128, 1], dt.float32, kind="ExternalInput")
    b1_d = nc.dram_tensor("b1v", [128, 1], dt.float32, kind="ExternalInput")
    out_d = nc.dram_tensor("out", [128, SR * W], dt.float32,
                           kind="ExternalOutput")

    DR = mybir.MatmulPerfMode.DoubleRow

    with tile.TileContext(nc) as tc:
        with tc.tile_pool(name="persist", bufs=1) as pp, \
             tc.tile_pool(name="mpool", bufs=2) as mpool, \
             tc.tile_pool(name="hsb", bufs=4) as hsbp, \
             tc.tile_pool(name="upool", bufs=2) as upool, \
             tc.tile_pool(name="hps", bufs=3, space="PSUM") as hps_pool, \
             tc.tile_pool(name="dps", bufs=1, space="PSUM") as dps_pool:

            s_sb = pp.tile([128, S_FREE], dt.float32)
            s8 = pp.tile([128, S_FREE], dt.float8e4)
            tw8 = pp.tile([128, 5 * 256], dt.float8e4)
            w2b = pp.tile([128, 2 * 256], dt.float8e4)
            b2r = pp.tile([128, 1], dt.float32)
            b1v = pp.tile([128, 1], dt.float32)

            # split the fp8 state load: the first half (rows read by the
            # first two chunk pairs) goes on the fast SP queue right after
            # the tap weights so step 0 starts ~1.4us earlier; the rest
            # rides the scalar queue behind the ACT table load.
            SPL = FRAME_OFF + 14 * FW
            nc.sync.dma_start(tw8[:], tw8_d[:])
            nc.sync.dma_start(s8[:, 0:SPL], s8_d[:, 0:SPL])
            nc.gpsimd.dma_start(s_sb[:], s_d[:])
            nc.scalar.dma_start(s8[:, SPL:S_FREE], s8_d[:, SPL:S_FREE])
            nc.sync.dma_start(b1v[:], b1_d[:])
            nc.sync.dma_start(w2b[:], w2b_d[:])
            nc.sync.dma_start(b2r[:], b2r_d[:])

            def halo_top():
                # top halos: sub g+1 frame row 0 <- sub g real row 23,
                # on the otherwise-idle gpsimd DMA queue (off critical path)
                for g in range(3):
                    nc.gpsimd.dma_start(
                        s8[32 * (g + 1):32 * (g + 1) + 24,
                           FRAME_OFF:FRAME_OFF + FW],
                        s8[32 * g:32 * g + 24,
                           FRAME_OFF + 24 * FW:FRAME_OFF + 24 * FW + FW],
                    )

            def halo_bot():
                # bottom halos: sub g frame row 25 <- sub g+1 real row 0
                for g in range(3):
                    nc.gpsimd.dma_start(
                        s8[32 * g:32 * g + 24,
                           FRAME_OFF + 25 * FW:FRAME_OFF + 25 * FW + FW],
                        s8[32 * (g + 1):32 * (g + 1) + 24,
                           FRAME_OFF + FW:FRAME_OFF + FW + FW],
                    )

            # DR slot plan: 5 matmuls per chunk cover the 9 taps — pairs
            # (0,1), (2,3), (5,6), (7,8) ride the two DR slots as two
            # shifted views (plain fp8, their quantization residuals are
            # small and mostly cancel), while the dominant center tap gets
            # the (A, B) compensated pair on a stride-0 view.
            # (dy, dx) of the first tap of each matmul and the pair stride:
            TAP_MMS = [
                ((0, 0), 2),            # taps (0,0)+(0,2)
                ((0, 1), 2 * FW),       # taps (0,1)+(2,1)
                ((1, 1), 0),            # center (A, B)
                ((1, 0), 2),            # taps (1,0)+(1,2)
                ((2, 0), 2),            # taps (2,0)+(2,2)
            ]

            def tap_group(k, p, split_relu=False):
                # 5-matmul fp8 DoubleRow chain for chunk pair k, slab pair
                # p, then relu+b1 (with the fp8 descale) into an fp8 h tile
                base = 64 * p
                hps = hps_pool.tile([128, 1024], dt.float32, tag="hps")
                for cc in range(2):
                    c = 2 * k + cc
                    for ti, ((dy, dx), stride) in enumerate(TAP_MMS):
                        off = FRAME_OFF + (3 * c + dy) * FW + dx - 1
                        nc.tensor.matmul(
                            hps[:, 512 * cc:512 * cc + NCH],
                            tw8[base:base + 56,
                                256 * ti:256 * ti + 256].rearrange(
                                    "p (two m) -> p two m", two=2),
                            _pair(s8[base:base + 56, off:off + NCH], stride),
                            start=(ti == 0), stop=(ti == 4),
                            tile_position=(base, 0),
                            perf_mode=DR,
                        )
                hsb = hsbp.tile([128, 2 * NCH], dt.float8e4, tag=f"hsb{p}")
                if split_relu:
                    # per-sub-chunk relu starts at the chain midpoint, so a
                    # layer2 that must run right after this group (the
                    # step-tail pipeline) does not wait a full relu latency
                    for cc in range(2):
                        nc.scalar.activation(
                            hsb[:, NCH * cc:NCH * cc + NCH],
                            hps[:, 512 * cc:512 * cc + NCH],
                            mybir.ActivationFunctionType.Relu,
                            bias=b1v[:, 0:1],
                            scale=SH8 / (SS * SW),
                        )
                else:
                    nc.scalar.activation(
                        hsb[:].rearrange("p (b x) -> p b x", x=NCH),
                        hps[:].rearrange("p (b x) -> p b x", b=2)[:, :, 0:NCH],
                        mybir.ActivationFunctionType.Relu,
                        bias=b1v[:, 0:1],
                        scale=SH8 / (SS * SW),
                    )
                return hsb

            def finish_chunk(k, hsb_list, m_sb, u_sb, fuse=None,
                             add_eng=None):
                # layer2 (fp8 DR, compensated), fire-mask, s update for
                # chunk pair k
                dps = dps_pool.tile([128, 1024], dt.float32, tag="dps")
                for cc in range(2):
                    for p in range(2):
                        # M padded to 128 (out rows 120-127 junk): DoubleRow
                        # requires the lhsT pair-dim step % 16 == 0
                        nc.tensor.matmul(
                            dps[0:128, 512 * cc:512 * cc + NCH],
                            w2b[:, 256 * p:256 * p + 256].rearrange(
                                "p (two m) -> p two m", two=2),
                            _pair0(hsb_list[p][:, NCH * cc:NCH * cc + NCH]),
                            start=(p == 0), stop=(p == 1),
                            perf_mode=DR,
                        )
                if apply_b2:
                    # delta += b2*SW2*SH8 (per-partition scalar), in psum
                    nc.vector.tensor_scalar_add(
                        dps[0:120].rearrange(
                            "p (b x) -> p b x", b=2)[:, :, 0:NCH],
                        dps[0:120].rearrange(
                            "p (b x) -> p b x", b=2)[:, :, 0:NCH],
                        b2r[0:120, 0:1],
                    )
                # u = m01 * delta / (SW2*SH8)
                nc.vector.scalar_tensor_tensor(
                    u_sb[0:120, 780 * k:780 * k + 780].rearrange(
                        "p (b x) -> p b x", x=NCH),
                    m_sb[0:120, 780 * k:780 * k + 780].rearrange(
                        "p (b x) -> p b x", x=NCH),
                    1.0 / (SW2 * SH8),
                    dps[0:120].rearrange("p (b x) -> p b x", b=2)[:, :, 0:NCH],
                    mybir.AluOpType.mult,
                    mybir.AluOpType.mult,
                )
                # fused critical shadow piece: s8 = fp8(s_old + u) straight
                # off u, skipping the wait on the fp32 master add
                if fuse is not None:
                    r0, r1 = fuse
                    fa = FRAME_OFF + FW + FW * r0
                    fn = FW * (r1 - r0 + 1)
                    nc.vector.tensor_add(
                        s8[0:120, fa:fa + fn],
                        s_sb[0:120, fa:fa + fn],
                        u_sb[0:120, FW * r0:FW * r0 + fn],
                    )
                # s += u (fp32 master); slack-tolerant chunks go on the
                # otherwise-idle gpsimd engine to unload the DVE
                a = FRAME_OFF + FW + 780 * k
                (add_eng or nc.vector).tensor_add(
                    s_sb[0:120, a:a + 780],
                    s_sb[0:120, a:a + 780],
                    u_sb[0:120, 780 * k:780 * k + 780],
                )

            def shadow_rows(r0, r1):
                # refresh fp8 shadow for real rows r0..r1 inclusive
                a = FRAME_OFF + FW + FW * r0
                n = FW * (r1 - r0 + 1)
                nc.vector.tensor_scalar_mul(
                    s8[0:120, a:a + n], s_sb[0:120, a:a + n], SS)

            # Static per-step schedule. Chunk-pair order (0, 3, 1, 2):
            # consecutive steps never start on a chunk adjacent to the
            # previous step's last chunk, layer2 of chunk j is delayed one
            # tap group so the PE never waits on ACT relu, and the shadow
            # refresh is split by row range so each piece is emitted only
            # after every reader of the old value (tap groups of adjacent
            # chunks, which overlap by one row) has been issued.
            # tiny warmup matmuls right after the weight load start the
            # cost model's PE p-state ramp clock during the state DMAs
            warm = dps_pool.tile([128, 1024], dt.float32, tag="dps")
            for _ in range(3):
                nc.tensor.matmul(warm[:, 0:128], tw8[0:56, 0:128],
                                 tw8[0:56, 128:256], start=True, stop=True)

            pending = None
            for t in range(steps * repeats):
                t = t % steps
                m_sb = mpool.tile([128, COMP], dt.float8e4, tag="m")
                nc.sync.dma_start(m_sb[:], m_d[t])
                u_sb = upool.tile([128, COMP], dt.float32, tag="u")

                hs = {}
                # it0 — the previous step's tail (finish of its chunk 2 and
                # the late shadow pieces) is emitted between this step's
                # first two tap groups, so its relu wait and DVE chain hide
                # under PE work instead of stalling the step boundary
                hs[0] = [tap_group(0, 0)]
                if pending is not None:
                    pending()
                hs[0].append(tap_group(0, 1))
                # it1
                hs[1] = [tap_group(1, 0)]
                finish_chunk(0, hs[0], m_sb, u_sb, add_eng=nc.gpsimd)
                hs[1].append(tap_group(1, 1))
                # it2 (chunk k's taps read rows 6k-2 .. 6k+6, so a shadow
                # piece for row r must follow the tap groups of every chunk
                # k with 6k-2 <= r <= 6k+6)
                hs[3] = [tap_group(3, 0)]
                finish_chunk(1, hs[1], m_sb, u_sb, add_eng=nc.gpsimd)
                shadow_rows(0, 4)
                shadow_rows(5, 9)
                hs[3].append(tap_group(3, 1))
                # it3
                hs[2] = [tap_group(2, 0)]
                finish_chunk(3, hs[3], m_sb, u_sb, fuse=(19, 23),
                             add_eng=nc.gpsimd)
                halo_top()
                halo_bot()
                hs[2].append(tap_group(2, 1))

                hs2, msb_t, usb_t = hs[2], m_sb, u_sb

                def pending(hs2=hs2, m_sb=msb_t, u_sb=usb_t):
                    # rows 12-17 are chunk 2's own (pre-add) so they ride
                    # the fused s8=fp8(s_old+u) path; rows 10-11 and 18
                    # belong to already-updated chunks, plain copies
                    finish_chunk(2, hs2, m_sb, u_sb, fuse=(12, 17),
                                 add_eng=nc.gpsimd)
                    shadow_rows(10, 11)
                    shadow_rows(18, 18)

            pending()

            # write back real pixels (frame rows 1..24, cols 1..128)
            a0 = FRAME_OFF + FW + 1
            nc.sync.dma_start(
                out_d[:].rearrange("p (r x) -> p r x", x=W),
                s_sb[:, a0:a0 + SR * FW].rearrange(
                    "p (r x) -> p r x", x=FW)[:, :, 0:W],
            )

    nc.compile()
    return nc


def _prep_weights(w1, b1, w2, b2):
    sx = np.array([[-1, 0, 1], [-2, 0, 2], [-1, 0, 1]], np.float32) / 8.0
    sy = sx.T.copy()
    ident = np.zeros((3, 3), np.float32)
    ident[1, 1] = 1.0
    # Weff[o, c, dy, dx]
    weff = (np.einsum("oc,yx->ocyx", w1[:, 0::3], ident)
            + np.einsum("oc,yx->ocyx", w1[:, 1::3], sx)
            + np.einsum("oc,yx->ocyx", w1[:, 2::3], sy)).astype(np.float32)

    # fp8 DR tap lhsT, 5 matmul slots per chunk (must match TAP_MMS):
    # slots 0,1,3,4 carry two different taps' fp8 weights; slot 2 carries
    # the center tap's (A, B) compensation pair. Block structure: rows
    # 0-23 -> h of even slab at out cols 0-63, rows 32-55 -> h of odd slab
    # at out cols 64-127 (same again at partition base 64).
    def q8w(dy, dx):
        return (weff[:, :, dy, dx].T * SW).astype(F8).astype(np.float32)

    cA = weff[:, :, 1, 1].T * SW
    cAq = cA.astype(F8).astype(np.float32)
    cBq = (cA - cAq).astype(F8).astype(np.float32)
    slot_pairs = [
        (q8w(0, 0), q8w(0, 2)),
        (q8w(0, 1), q8w(2, 1)),
        (cAq, cBq),
        (q8w(1, 0), q8w(1, 2)),
        (q8w(2, 0), q8w(2, 2)),
    ]
    tw8 = np.zeros((128, 5 * 256), np.float32)
    for ti, (w0, w1_) in enumerate(slot_pairs):
        for p in range(2):
            base = 64 * p
            o = 256 * ti
            tw8[base:base + 24, o:o + 64] = w0
            tw8[base + 32:base + 56, o + 64:o + 128] = w0
            tw8[base:base + 24, o + 128:o + 192] = w1_
            tw8[base + 32:base + 56, o + 192:o + 256] = w1_
    tw8 = tw8.astype(F8)

    # layer2 lhsT per pair: K=128 (both h halves), M=120 with 24-col blocks
    # placing each slab's delta on its partition quadrant. fp8 DR with (A,B)
    # compensation: slot0 = A = fp8(w2*SW2), slot1 = B = fp8(w2*SW2 - A).
    w2s = w2.T * SW2
    w2A = w2s.astype(F8).astype(np.float32)
    w2B = (w2s - w2A).astype(F8).astype(np.float32)
    w2b = np.zeros((128, 2 * 256), np.float32)
    for p in range(2):
        ge, go = 2 * p, 2 * p + 1
        o = 256 * p
        w2b[0:64, o + 32 * ge:o + 32 * ge + 24] = w2A
        w2b[64:128, o + 32 * go:o + 32 * go + 24] = w2A
        w2b[0:64, o + 128 + 32 * ge:o + 128 + 32 * ge + 24] = w2B
        w2b[64:128, o + 128 + 32 * go:o + 128 + 32 * go + 24] = w2B
    w2b = w2b.astype(F8)

    b2r = np.zeros((128, 1), np.float32)
    b1v = np.zeros((128, 1), np.float32)
    for g in range(4):
        b2r[32 * g:32 * g + 24, 0] = b2 * SW2 * SH8
    b1v[0:64, 0] = b1
    b1v[64:128, 0] = b1
    return tw8, w2b, b2r, b1v


def _prep_state(state):
    """state (B, C, H, W) -> per-core [128, S_FREE] framed slabs (+fp8)."""
    bufs = []
    for core in range(N_CORES):
        b = core // 2
        top = (core % 2) == 0
        r0 = 0 if top else H - SH
        buf = np.zeros((128, S_FREE), np.float32)
        for ch in range(C):
            full = np.zeros((SH + 2, FW), np.float32)
            full[1:SH + 1, 1:W + 1] = state[b, ch, r0:r0 + SH, :]
            if r0 > 0:
                full[0, 1:W + 1] = state[b, ch, r0 - 1, :]
            if r0 + SH < H:
                full[SH + 1, 1:W + 1] = state[b, ch, r0 + SH, :]
            for g in range(4):
                fr = full[g * SR:g * SR + FR, :]
                buf[32 * g + ch, FRAME_OFF:FRAME_OFF + FRAME] = fr.reshape(-1)
        bufs.append((buf, (buf * SS).astype(F8)))
    return bufs


def _prep_masks(masks):
    """masks (S, B, 1, H, W) -> per-core [S, 128, COMP] fp8 {0,1} fire."""
    S = masks.shape[0]
    bufs = []
    for core in range(N_CORES):
        b = core // 2
        top = (core % 2) == 0
        r0 = 0 if top else H - SH
        mb = np.zeros((S, 128, COMP), F8)
        fire = (masks[:, b, 0, r0:r0 + SH, :] < FIRE_RATE).astype(np.float32)
        mrows = np.zeros((S, SH, FW), np.float32)
        mrows[:, :, 1:W + 1] = fire
        for g in range(4):
            seg = mrows[:, g * SR:(g + 1) * SR, :].reshape(S, COMP)
            mb[:, 32 * g:32 * g + C, :] = seg[:, None, :].astype(F8)
        bufs.append(mb)
    return bufs


def kernel(state, w1, b1, w2, b2, masks):
    state = np.asarray(state)
    w1, b1 = np.asarray(w1), np.asarray(b1)
    w2, b2 = np.asarray(w2), np.asarray(b2)
    masks = np.asarray(masks)
    import os as _os
    steps = masks.shape[0]
    apply_b2 = bool(np.any(b2 != 0))
    repeats = int(_os.environ.get("NCA_REPEAT", "1"))
    key = ("prog", steps, apply_b2, repeats)
    if key not in _cache:
        _cache[key] = _build_program(steps, apply_b2, repeats)
    nc = _cache[key]

    from concourse.bass_utils import run_bass_kernel_spmd

    tw8, w2b, b2r, b1v = _prep_weights(w1, b1, w2, b2)
    s_bufs = _prep_state(state)
    m_bufs = _prep_masks(masks)

    in_maps = []
    for core in range(N_CORES):
        in_maps.append({
            "s0": s_bufs[core][0],
            "s80": s_bufs[core][1],
            "masks": m_bufs[core],
            "tw8": tw8,
            "w2b": w2b,
            "b2r": b2r,
            "b1v": b1v,
        })

    import os
    trace = bool(os.environ.get("NCA_TRACE"))
    kw = {}
    if trace:
        kw["trace"] = True
        if os.environ.get("NCA_TRACE_DIR"):
            kw["tmpdir"] = os.environ["NCA_TRACE_DIR"]
    res = run_bass_kernel_spmd(nc, in_maps, list(range(N_CORES)), **kw)
    global LAST_EXEC_NS
    LAST_EXEC_NS = res.exec_time_ns

    out = np.zeros((B, C, H, W), np.float32)
    for core in range(N_CORES):
        o = res.results[core]["out"]  # [128, SR*W]
        b = core // 2
        top = (core % 2) == 0
        r0 = 0 if top else H - SH
        own0 = 0 if top else H // 2
        for g in range(4):
            rows = o[32 * g:32 * g + 24].reshape(C, SR, W)
            g0 = r0 + g * SR
            lo = max(g0, own0)
            hi = min(g0 + SR, own0 + H // 2)
            if lo < hi:
                out[b, :, lo:hi, :] = rows[:, lo - g0:hi - g0, :]
    return out
